# revision 1
# baseline (speedup 1.0000x reference)
"""Trainium2 Bass kernel: fused QKV + RoPE + causal/windowed GQA attention + output proj.

Sharding: tensor-parallel by head across 8 cores. Core c owns Q-heads
4c..4c+3 and KV-group c (matching repeat_interleave grouping), plus the
512 w_o columns for those heads. Each core computes a full-shape partial
of the final output (contraction over its 512 attention-output dims);
the host sums the 8 partials. No device collectives.

Dataflow is in transposed-activation space so every matmul contracts on
the partition dim; matmul operands are bf16 (full PE rate), all
accumulation/softmax math is fp32 in PSUM:
  P1: qkvT[e, tok] = w_qkvT^T @ xT           (xT pre-transposed on host)
  P2: ST[k, q] = kT^T @ qT  -> exp -> PV and row-sum both as matmuls
      (softmax normalization via reciprocal + partition_broadcast)
  P3: out_partial[tok, e] = outT^T @ w_oT    (outT kept SBUF-resident)

RoPE is applied on interleaved even/odd pairs via a DVE stream_shuffle
pair swap and a sign-folded sin table.
"""

import math
import sys
from contextlib import ExitStack

import numpy as np

sys.path.insert(0, "/opt/trn_rl_repo")

import ml_dtypes

BF16NP = ml_dtypes.bfloat16

import concourse.bass as bass
import concourse.mybir as mybir
import concourse.tile as tile
from concourse import bacc

F32 = mybir.dt.float32
F32R = mybir.dt.float32r
BF16 = mybir.dt.bfloat16

B, T, D = 2, 2048, 4096
H, G, HD = 32, 8, 128
THETA = 10000.0
NCORES = 8
HL = H // NCORES            # 4 local q heads
TOK = B * T                 # 4096
QROWS = HL * HD             # 512 local q rows
E = QROWS + 2 * HD          # 768 local qkv rows
SCALE = 1.0 / math.sqrt(HD)

TOKG = 256                  # P1 token-group width
NTOKG = TOK // TOKG
NDC = D // 128              # 32 contraction chunks
NE = E // 128               # 6 qkv row chunks
QG = 512                    # P2 query-group width (within batch)
NQG = T // QG               # 4
NKC = T // 128              # 16 key chunks per batch
MASK_NEG = -1.0e30


def _mask_plan(window: int):
    """Per (qgroup, kchunk): 'skip', 'full', or a mask-key (delta-based)."""
    plan = {}
    keys = {}
    for g in range(NQG):
        for kc in range(NKC):
            i_min, i_max = QG * g, QG * g + QG - 1
            j_min, j_max = 128 * kc, 128 * kc + 127
            if j_min > i_max or (i_min - j_max) >= window:
                plan[(g, kc)] = ("skip", None)
            elif j_max <= i_min and (i_max - j_min) < window:
                plan[(g, kc)] = ("full", None)
            else:
                key = QG * g - 128 * kc
                if key not in keys:
                    keys[key] = len(keys)
                plan[(g, kc)] = ("mask", keys[key])
    return plan, keys


def _build_masks(window: int, keys: dict) -> np.ndarray:
    n = max(1, len(keys))
    m = np.zeros((n, 128, QG), dtype=np.float32)  # cast to bf16 in kernel()
    for key, idx in keys.items():
        # i = key + 128*kc ... i - j = key + qq - kk
        qq = np.arange(QG)[None, :]
        kk = np.arange(128)[:, None]
        diff = key + qq - kk          # i - j
        vis = (diff >= 0) & (diff < window)
        m[idx] = np.where(vis, 1.0, 0.0)
    return m


PAIRSWAP = [i ^ 1 for i in range(32)]


def _rope_ops(nc, pool, dst, src, cos_ap, sin_ap):
    """Interleaved-pair RoPE: dst = src*cos + pairswap(src)*signed_sin.

    cos_ap rows (2i, 2i+1) hold cos_i; sin_ap rows hold (-sin_i, +sin_i).
    src may alias dst (in-place).
    """
    W = dst.shape[-1]
    sw = pool.tile([128, W], BF16, tag="rope_sw")
    tmp = pool.tile([128, W], BF16, tag="rope_tmp")
    qc = pool.tile([128, W], BF16, tag="rope_qc")
    mult = mybir.AluOpType.mult
    nc.vector.stream_shuffle(sw, src, PAIRSWAP)
    nc.vector.tensor_tensor(tmp, sw, sin_ap, mult)
    nc.vector.tensor_tensor(qc, src, cos_ap, mult)
    nc.vector.tensor_tensor(dst, qc, tmp, mybir.AluOpType.add)


class _PhaseStop(Exception):
    pass


def build_nc(window: int, phases=(1, 2, 3)):
    plan, keys = _mask_plan(window)
    nmask = max(1, len(keys))

    nc = bacc.Bacc()
    xT_d = nc.dram_tensor("xT", [D, TOK], BF16, kind="ExternalInput")
    wqkvT_d = nc.dram_tensor("wqkvT", [D, E], BF16, kind="ExternalInput")
    woT_d = nc.dram_tensor("woT", [QROWS, D], BF16, kind="ExternalInput")
    cos_d = nc.dram_tensor("cosH", [128, T], BF16, kind="ExternalInput")
    sin_d = nc.dram_tensor("sinH", [128, T], BF16, kind="ExternalInput")
    masks_d = nc.dram_tensor("masks", [nmask, 128, QG], BF16, kind="ExternalInput")
    ident_d = nc.dram_tensor("ident", [128, 128], BF16, kind="ExternalInput")
    out_d = nc.dram_tensor("out", [TOK, D], F32, kind="ExternalOutput")

    with ExitStack() as octx:
        tc = octx.enter_context(tile.TileContext(nc))
        qkvp = octx.enter_context(tc.tile_pool(name="qkvT", bufs=1))
        qkvT_sb = [qkvp.tile([128, TOK], BF16, tag=f"qkv{e}", name=f"qkv{e}")
                   for e in range(NE)]

        # ---------------- P1: qkvT = w^T @ xT ----------------
        if 1 in phases:
         with ExitStack() as ctx:
            wpool = ctx.enter_context(tc.tile_pool(name="w1", bufs=1))
            xpool = ctx.enter_context(tc.tile_pool(name="x1", bufs=3))
            ppool = ctx.enter_context(tc.tile_pool(name="ps1", bufs=6, space="PSUM"))

            wsb = wpool.tile([128, NDC, E], BF16)
            wq_r = wqkvT_d[:].rearrange("(dc p) e -> p dc e", p=128)
            for dc in range(NDC):
                nc.sync.dma_start(out=wsb[:, dc, :], in_=wq_r[:, dc, :])
            for g in range(NTOKG):
                xsb = xpool.tile([128, NDC, TOKG], BF16, tag="xslab")
                x_r = xT_d[:, g * TOKG:(g + 1) * TOKG].rearrange(
                    "(dc p) t -> p dc t", p=128)
                for dq in range(4):
                    nc.sync.dma_start(out=xsb[:, dq * 8:(dq + 1) * 8, :],
                                      in_=x_r[:, dq * 8:(dq + 1) * 8, :])
                for e in range(NE):
                    ps = ppool.tile([128, TOKG], F32, tag="p1")
                    for dc in range(NDC):
                        nc.tensor.matmul(
                            ps,
                            lhsT=wsb[:, dc, e * 128:(e + 1) * 128],
                            rhs=xsb[:, dc, :],
                            start=(dc == 0), stop=(dc == NDC - 1))
                    # fold softmax 1/sqrt(HD) into q rows; evict into the
                    # SBUF-resident qkvT directly
                    nc.scalar.mul(
                        qkvT_sb[e][:, g * TOKG:(g + 1) * TOKG], ps,
                        SCALE if e < HL else 1.0)

        # ---------------- P2: attention ----------------
        if 2 in phases:
            # outT survives P2 -> P3: allocate after P1's pools are released.
            opool = octx.enter_context(tc.tile_pool(name="outT", bufs=1))
            outT = [opool.tile([128, TOK], BF16, tag=f"outT{i}", name=f"outT{i}")
                    for i in range(HL)]
            p2ctx = ExitStack()
            kpool = p2ctx.enter_context(tc.tile_pool(name="kv", bufs=1))
            ksb = qkvT_sb[HL]
            vsb = kpool.tile([128, TOK // 128, 128], BF16, tag="v")
            cos_sb = kpool.tile([128, T], BF16, tag="cos")
            sin_sb = kpool.tile([128, T], BF16, tag="sin")
            ones_sb = kpool.tile([128, 1], BF16, tag="ones")
            mask_sb = kpool.tile([128, nmask, QG], BF16, tag="masks")

            nc.sync.dma_start(out=cos_sb, in_=cos_d[:])
            nc.sync.dma_start(out=sin_sb, in_=sin_d[:])
            nc.sync.dma_start(
                out=mask_sb, in_=masks_d[:].rearrange("n p q -> p n q"))
            nc.vector.memset(ones_sb, 1.0)

            with ExitStack() as ctx:
                sc0 = ctx.enter_context(tc.tile_pool(name="p2a", bufs=1))
                pt0 = ctx.enter_context(tc.tile_pool(name="p2aps", bufs=2, space="PSUM"))
                ident = sc0.tile([128, 128], BF16, tag="ident")
                nc.sync.dma_start(out=ident, in_=ident_d[:])
                vT = qkvT_sb[HL + 1]
                for tc32 in range(TOK // 128):
                    pst = pt0.tile([128, 128], BF16, tag="tr")
                    nc.tensor.transpose(
                        pst, vT[:, tc32 * 128:(tc32 + 1) * 128], ident)
                    nc.scalar.copy(vsb[:, tc32, :], pst)
                # RoPE on k (per batch)
                for b in range(B):
                    kslice = ksb[:, b * T:(b + 1) * T]
                    _rope_ops(nc, sc0, kslice, kslice, cos_sb, sin_sb)

            # P2 attention interleaved with P3 (output projection): after the 4
            # head-instances of a (batch, q-group) window finish, that window's
            # outT columns are final, so its P3 tiles are emitted immediately —
            # the Tile scheduler uses them to fill PE gaps in later P2 windows.
            with ExitStack() as ctx:
                qpool = ctx.enter_context(tc.tile_pool(name="q2", bufs=4))
                spool = ctx.enter_context(tc.tile_pool(name="sc2", bufs=4))
                estp = ctx.enter_context(tc.tile_pool(name="est", bufs=6))
                wpool = ctx.enter_context(tc.tile_pool(name="wo", bufs=1))
                panp = ctx.enter_context(tc.tile_pool(name="pan", bufs=2))
                stps = ctx.enter_context(tc.tile_pool(name="stps", bufs=3, space="PSUM"))
                rps = ctx.enter_context(tc.tile_pool(name="rps", bufs=1, space="PSUM"))
                ops = ctx.enter_context(tc.tile_pool(name="ops", bufs=2, space="PSUM"))
                pps = ctx.enter_context(tc.tile_pool(name="ps3", bufs=2, space="PSUM"))

                wo = []
                for dc in range(HL):
                    w = wpool.tile([128, D], BF16, tag=f"wo{dc}", name=f"wo{dc}")
                    nc.sync.dma_start(
                        out=w, in_=woT_d[dc * 128:(dc + 1) * 128, :])
                    wo.append(w)

                for b in range(B):
                    for g in range(NQG):
                        for hh in range(HL):
                            qsb = qpool.tile([128, QG], BF16, tag="q")
                            _rope_ops(nc, qpool, qsb,
                                      qkvT_sb[hh][:, b * T + g * QG:
                                                  b * T + (g + 1) * QG],
                                      cos_sb[:, g * QG:(g + 1) * QG],
                                      sin_sb[:, g * QG:(g + 1) * QG])
                            vis = [(kc, plan[(g, kc)]) for kc in range(NKC)
                                   if plan[(g, kc)][0] != "skip"]
                            r_ps = rps.tile([1, QG], F32, tag="r")
                            o_ps = ops.tile([128, QG], F32, tag="o")
                            for idx, (kc, (kind, mid)) in enumerate(vis):
                                # visible query subrange of this key chunk:
                                # qq >= -aoff (causal), qq < w - aoff + 127
                                aoff = QG * g - 128 * kc
                                qlo = max(0, -aoff)
                                qhi = min(QG, window - aoff + 127)
                                qsl = slice(qlo, qhi)
                                st = stps.tile([128, QG], F32, tag="st")
                                nc.tensor.matmul(
                                    st[:, qsl],
                                    lhsT=ksb[:, b * T + kc * 128:
                                             b * T + (kc + 1) * 128],
                                    rhs=qsb[:, qsl],
                                    start=True, stop=True)
                                est = estp.tile([128, QG], BF16, tag="est")
                                nc.scalar.activation(
                                    est[:, qsl], st[:, qsl],
                                    mybir.ActivationFunctionType.Exp)
                                if kind == "mask":
                                    nc.vector.tensor_tensor(
                                        est[:, qsl], est[:, qsl],
                                        mask_sb[:, mid, qsl],
                                        mybir.AluOpType.mult)
                                last = idx == len(vis) - 1
                                nc.tensor.matmul(
                                    r_ps[:, qsl], lhsT=ones_sb,
                                    rhs=est[:, qsl],
                                    start=(idx == 0), stop=last)
                                nc.tensor.matmul(
                                    o_ps[:, qsl],
                                    lhsT=vsb[:, b * NKC + kc, :],
                                    rhs=est[:, qsl],
                                    start=(idx == 0), stop=last)
                            rrec = spool.tile([1, QG], F32, tag="rrec")
                            nc.vector.reciprocal(rrec, r_ps)
                            rb = spool.tile([128, QG], F32, tag="rb")
                            nc.gpsimd.partition_broadcast(rb, rrec)
                            nc.vector.tensor_tensor(
                                outT[hh][:, b * T + g * QG: b * T + (g + 1) * QG],
                                o_ps, rb, mybir.AluOpType.mult)

                        # P3 for this window's 4 token chunks
                        for tloc in range(QG // 128):
                            tch = (b * T + g * QG) // 128 + tloc
                            panel = panp.tile([128, D], F32, tag="panel")
                            for et in range(D // 512):
                                ps = pps.tile([128, 512], F32, tag="p3")
                                for dc in range(HL):
                                    nc.tensor.matmul(
                                        ps,
                                        lhsT=outT[dc][:,
                                                      tch * 128:(tch + 1) * 128],
                                        rhs=wo[dc][:, et * 512:(et + 1) * 512],
                                        start=(dc == 0), stop=(dc == HL - 1))
                                nc.scalar.copy(
                                    panel[:, et * 512:(et + 1) * 512], ps)
                            nc.sync.dma_start(
                                out=out_d[tch * 128:(tch + 1) * 128, :], in_=panel)

            p2ctx.close()

    nc.finalize()
    return nc, nmask


_CACHE = {}


def _get_nc(window: int):
    if window not in _CACHE:
        _CACHE[window] = build_nc(window)
    return _CACHE[window]


LAST_RESULTS = None


def kernel(x, w_qkv, w_o, window_size, _trace=False):
    window = int(window_size)
    nc, nmask = _get_nc(window)
    _, keys = _mask_plan(window)
    masks = _build_masks(window, keys)

    xT = np.ascontiguousarray(x.reshape(TOK, D).T).astype(BF16NP)

    inv = 1.0 / (THETA ** (np.arange(0, HD, 2, dtype=np.float64) / HD))
    freqs = np.arange(T, dtype=np.float64)[:, None] * inv[None, :]  # [T, 64]
    cosH = np.repeat(np.cos(freqs).T, 2, axis=0).astype(BF16NP)  # [128, T]
    sign = np.where(np.arange(HD) % 2 == 0, -1.0, 1.0)[:, None]
    sinH = (np.repeat(np.sin(freqs).T, 2, axis=0) * sign).astype(BF16NP)
    ident = np.eye(128).astype(BF16NP)

    in_maps = []
    for c in range(NCORES):
        wq = w_qkv[QROWS * c:QROWS * (c + 1)]
        wk = w_qkv[H * HD + HD * c: H * HD + HD * (c + 1)]
        wv = w_qkv[H * HD + G * HD + HD * c: H * HD + G * HD + HD * (c + 1)]
        wqkvT = np.ascontiguousarray(
            np.concatenate([wq, wk, wv], axis=0).T).astype(BF16NP)
        woT = np.ascontiguousarray(
            w_o[:, QROWS * c:QROWS * (c + 1)].T).astype(BF16NP)
        in_maps.append({
            "xT": xT, "wqkvT": wqkvT, "woT": woT,
            "cosH": cosH, "sinH": sinH, "masks": masks.astype(BF16NP),
            "ident": ident,
        })

    from concourse.bass_utils import run_bass_kernel_spmd
    res = run_bass_kernel_spmd(nc, in_maps, core_ids=list(range(NCORES)),
                               trace=_trace)
    global LAST_RESULTS
    LAST_RESULTS = res
    acc = res.results[0]["out"].astype(np.float32).copy()
    for c in range(1, NCORES):
        acc += res.results[c]["out"]
    return acc.reshape(B, T, D)



# revision 7
# speedup vs baseline: 1.1032x; 1.1032x over previous
"""Trainium2 Bass kernel: fused QKV + RoPE + causal/windowed GQA attention + output proj.

Sharding: tensor-parallel by head across 8 cores. Core c owns Q-heads
4c..4c+3 and KV-group c (matching repeat_interleave grouping), plus the
512 w_o columns for those heads. Each core computes a full-shape partial
of the final output (contraction over its 512 attention-output dims);
the host sums the 8 partials. No device collectives.

All activations/weights are fp16 (full PE rate, 2x DVE modes, ~4x better
mantissa than bf16); accumulation/softmax math is fp32 in PSUM.

  P1: qkvT[e, tok] = w_qkvT^T @ xT   (xT pre-transposed on host; the v
      rows are instead produced pre-transposed as v[tok, vd] by swapping
      the stationary operand, so attention needs no PE transposes).
      RoPE (q and k, in place) runs per token-group in P1's shadow.
  P2: ST[k, q] = kT^T @ qT -> exp -> PV matmul; the softmax denominator
      is built by DVE-accumulating the exp tiles and one gpsimd
      partition_all_reduce per window (no PE rowsum matmuls).
  P3: out_partial[tok, e] = outT^T @ w_oT    (outT kept SBUF-resident)

RoPE is applied on interleaved even/odd pairs via a DVE stream_shuffle
pair swap and a sign-folded sin table; the softmax 1/sqrt(HD) scale is
folded into the q rows of w_qkv on the host.
"""

import math
import sys
from contextlib import ExitStack

import numpy as np

sys.path.insert(0, "/opt/trn_rl_repo")

F16NP = np.float16

import concourse.bass as bass
import concourse.bass_isa as bass_isa
import concourse.mybir as mybir
import concourse.tile as tile
from concourse import bacc

F32 = mybir.dt.float32
F16 = mybir.dt.float16

B, T, D = 2, 2048, 4096
H, G, HD = 32, 8, 128
THETA = 10000.0
NCORES = 8
HL = H // NCORES            # 4 local q heads
TOK = B * T                 # 4096
QROWS = HL * HD             # 512 local q rows
E = QROWS + 2 * HD          # 768 local qkv rows
SCALE = 1.0 / math.sqrt(HD)

TOKG = 256                  # P1 token-group width
NTOKG = TOK // TOKG
NDC = D // 128              # 32 contraction chunks
NE = E // 128               # 6 qkv row chunks (last = v, handled transposed)
QG = 512                    # P2 query-group width (within batch)
NQG = T // QG               # 4
NKC = T // 128              # 16 key chunks per batch


def _mask_plan(window: int):
    """Per (qgroup, kchunk): 'skip', 'full', or a mask-key (delta-based)."""
    plan = {}
    keys = {}
    for g in range(NQG):
        for kc in range(NKC):
            i_min, i_max = QG * g, QG * g + QG - 1
            j_min, j_max = 128 * kc, 128 * kc + 127
            if j_min > i_max or (i_min - j_max) >= window:
                plan[(g, kc)] = ("skip", None)
            elif j_max <= i_min and (i_max - j_min) < window:
                plan[(g, kc)] = ("full", None)
            else:
                key = QG * g - 128 * kc
                if key not in keys:
                    keys[key] = len(keys)
                plan[(g, kc)] = ("mask", keys[key])
    return plan, keys


def _build_masks(window: int, keys: dict) -> np.ndarray:
    n = max(1, len(keys))
    m = np.zeros((n, 128, QG), dtype=np.float32)
    for key, idx in keys.items():
        qq = np.arange(QG)[None, :]
        kk = np.arange(128)[:, None]
        diff = key + qq - kk          # i - j
        vis = (diff >= 0) & (diff < window)
        m[idx] = np.where(vis, 1.0, 0.0)
    return m


PAIRSWAP = [i ^ 1 for i in range(32)]


def _rope_ops(nc, pool, dst, src, cos_ap, sin_ap):
    """Interleaved-pair RoPE: dst = src*cos + pairswap(src)*signed_sin.

    cos_ap rows (2i, 2i+1) hold cos_i; sin_ap rows hold (-sin_i, +sin_i).
    src may alias dst (in-place).
    """
    W = dst.shape[-1]
    sw = pool.tile([128, W], F16, tag="rope_sw")
    tmp = pool.tile([128, W], F16, tag="rope_tmp")
    qc = pool.tile([128, W], F16, tag="rope_qc")
    mult = mybir.AluOpType.mult
    nc.vector.stream_shuffle(sw, src, PAIRSWAP)
    nc.vector.tensor_tensor(tmp, sw, sin_ap, mult)
    nc.vector.tensor_tensor(qc, src, cos_ap, mult)
    nc.vector.tensor_tensor(dst, qc, tmp, mybir.AluOpType.add)


def build_nc(window: int):
    plan, keys = _mask_plan(window)
    nmask = max(1, len(keys))
    add = mybir.AluOpType.add
    mult = mybir.AluOpType.mult

    nc = bacc.Bacc()
    xT_d = nc.dram_tensor("xT", [D, TOK], F16, kind="ExternalInput")
    wqkvT_d = nc.dram_tensor("wqkvT", [D, E], F16, kind="ExternalInput")
    woT_d = nc.dram_tensor("woT", [QROWS, D], F16, kind="ExternalInput")
    cos_d = nc.dram_tensor("cosH", [128, T], F16, kind="ExternalInput")
    sin_d = nc.dram_tensor("sinH", [128, T], F16, kind="ExternalInput")
    masks_d = nc.dram_tensor("masks", [nmask, 128, QG], F16, kind="ExternalInput")
    out_d = nc.dram_tensor("out", [TOK, D], F16, kind="ExternalOutput")

    with ExitStack() as octx:
        tc = octx.enter_context(tile.TileContext(nc))
        # Long-lived SBUF: qkv rows (q heads + k), v (pre-transposed),
        # attention outputs, rope tables, masks.
        qkvp = octx.enter_context(tc.tile_pool(name="qkvT", bufs=1))
        qkvT_sb = [qkvp.tile([128, TOK], F16, tag=f"qkv{e}", name=f"qkv{e}")
                   for e in range(NE - 1)]
        ksb = qkvT_sb[HL]
        vsb = qkvp.tile([128, TOK // 128, 128], F16, tag="v", name="v")
        cos_sb = qkvp.tile([128, T], F16, tag="cos")
        sin_sb = qkvp.tile([128, T], F16, tag="sin")
        mask_sb = qkvp.tile([128, nmask, QG], F16, tag="masks")
        opool = octx.enter_context(tc.tile_pool(name="outT", bufs=1))
        outT = [opool.tile([128, TOK], F16, tag=f"outT{i}", name=f"outT{i}")
                for i in range(HL)]
        ropep = octx.enter_context(tc.tile_pool(name="rope", bufs=3))

        nc.sync.dma_start(out=cos_sb, in_=cos_d[:])
        nc.sync.dma_start(out=sin_sb, in_=sin_d[:])
        nc.sync.dma_start(out=mask_sb, in_=masks_d[:].rearrange("n p q -> p n q"))

        # ---------------- P1: qkvT = w^T @ xT (+ in-place RoPE) -------------
        with ExitStack() as ctx:
            wpool = ctx.enter_context(tc.tile_pool(name="w1", bufs=1))
            xpool = ctx.enter_context(tc.tile_pool(name="x1", bufs=2))
            ppool = ctx.enter_context(tc.tile_pool(name="ps1", bufs=1, space="PSUM"))
            vpps = ctx.enter_context(tc.tile_pool(name="ps1v", bufs=1, space="PSUM"))

            wsb = wpool.tile([128, NDC, E], F16)
            wq_r = wqkvT_d[:].rearrange("(dc p) e -> p dc e", p=128)

            def x_slab(g, interleave_w=False):
                # First-group slab: interleave the x quarter-DMAs with the
                # weight stream so the dc-major matmul flow starts ~2us in
                # instead of waiting out the full 6.3MB weight transfer.
                xsb = xpool.tile([128, NDC, TOKG], F16, tag="xslab")
                x_r = xT_d[:, g * TOKG:(g + 1) * TOKG].rearrange(
                    "(dc p) t -> p dc t", p=128)
                for dq in range(4):
                    nc.sync.dma_start(out=xsb[:, dq * 8:(dq + 1) * 8, :],
                                      in_=x_r[:, dq * 8:(dq + 1) * 8, :])
                    if interleave_w:
                        for dc in range(dq * 8, dq * 8 + 8):
                            nc.sync.dma_start(out=wsb[:, dc, :],
                                              in_=wq_r[:, dc, :])
                return xsb

            xsb_next = x_slab(0, interleave_w=True)

            for g in range(NTOKG):
                xsb = xsb_next
                if g + 1 < NTOKG:
                    xsb_next = x_slab(g + 1)
                gt0 = g * TOKG
                bpos = gt0 % T            # position within batch
                cs = cos_sb[:, bpos:bpos + TOKG]
                sn = sin_sb[:, bpos:bpos + TOKG]
                # dc-major: all 7 accumulation chains advance together, so
                # the PE consumes (w, x) chunks in DMA-arrival order.
                pse = [ppool.tile([128, TOKG], F32, tag=f"p1_{e}",
                                  name=f"pse{g}_{e}")
                       for e in range(NE - 1)]
                psv = [vpps.tile([128, 128], F32, tag=f"p1v_{tl}",
                                 name=f"psv{g}_{tl}")
                       for tl in range(TOKG // 128)]
                for dc in range(NDC):
                    se, st = (dc == 0), (dc == NDC - 1)
                    for e in range(NE - 1):
                        nc.tensor.matmul(
                            pse[e],
                            lhsT=wsb[:, dc, e * 128:(e + 1) * 128],
                            rhs=xsb[:, dc, :],
                            start=se, stop=st)
                    # v, produced directly as [token, vdim] (x as stationary)
                    for tl in range(TOKG // 128):
                        nc.tensor.matmul(
                            psv[tl],
                            lhsT=xsb[:, dc, tl * 128:(tl + 1) * 128],
                            rhs=wsb[:, dc, QROWS + HD:E],
                            start=se, stop=st)
                for e in range(NE - 1):
                    dst = qkvT_sb[e][:, gt0:gt0 + TOKG]
                    nc.scalar.copy(dst, pse[e])
                    # RoPE in place (q scale folded into w_qkv on host)
                    _rope_ops(nc, ropep, dst, dst, cs, sn)
                for tl in range(TOKG // 128):
                    nc.scalar.copy(vsb[:, gt0 // 128 + tl, :], psv[tl])

        # ---------------- P2 attention + P3 output projection ----------------
        # P2 windows interleave with P3 (emitted per (b, g) right after the 4
        # head-instances finish); the Tile scheduler uses P3 tiles to fill PE
        # gaps in later P2 windows.
        with ExitStack() as ctx:
            estp = ctx.enter_context(tc.tile_pool(name="est", bufs=6))
            accp = ctx.enter_context(tc.tile_pool(name="acc", bufs=3))
            dnp = ctx.enter_context(tc.tile_pool(name="dn", bufs=2))
            rcp = ctx.enter_context(tc.tile_pool(name="rc", bufs=2))
            wpool = ctx.enter_context(tc.tile_pool(name="wo", bufs=1))
            panp = ctx.enter_context(tc.tile_pool(name="pan", bufs=2))
            stps = ctx.enter_context(tc.tile_pool(name="stps", bufs=2, space="PSUM"))
            ops = ctx.enter_context(tc.tile_pool(name="ops", bufs=2, space="PSUM"))
            pps = ctx.enter_context(tc.tile_pool(name="ps3", bufs=4, space="PSUM"))

            wo = []
            for dc in range(HL):
                w = wpool.tile([128, D], F16, tag=f"wo{dc}", name=f"wo{dc}")
                nc.sync.dma_start(
                    out=w, in_=woT_d[dc * 128:(dc + 1) * 128, :])
                wo.append(w)

            for b in range(B):
                for g in range(NQG):
                    for hh in range(HL):
                        vis = [(kc, plan[(g, kc)]) for kc in range(NKC)
                               if plan[(g, kc)][0] != "skip"]
                        qbase = b * T + g * QG
                        o_ps = ops.tile([128, QG], F32, tag="o")
                        acc = accp.tile([128, QG], F16, tag="acc")
                        # visible q subranges shrink as kc grows (causal), so
                        # chunk 0 spans every column later chunks touch; if it
                        # doesn't span [0, QG) (tiny sliding window), zero acc
                        # and add every chunk instead.
                        aoff0 = QG * g - 128 * vis[0][0]
                        full0 = (max(0, -aoff0) == 0
                                 and min(QG, window - aoff0 + 127) == QG)
                        if not full0:
                            nc.vector.memset(acc, 0.0)
                        for idx, (kc, (kind, mid)) in enumerate(vis):
                            aoff = QG * g - 128 * kc
                            qlo = max(0, -aoff)
                            qhi = min(QG, window - aoff + 127)
                            qsl = slice(qlo, qhi)
                            st = stps.tile([128, QG], F32, tag="st")
                            nc.tensor.matmul(
                                st[:, qsl],
                                lhsT=ksb[:, b * T + kc * 128:
                                         b * T + (kc + 1) * 128],
                                rhs=qkvT_sb[hh][:, qbase + qlo:qbase + qhi],
                                start=True, stop=True)
                            into_acc = full0 and idx == 0
                            est = acc if into_acc else estp.tile(
                                [128, QG], F16, tag="est")
                            nc.scalar.activation(
                                est[:, qsl], st[:, qsl],
                                mybir.ActivationFunctionType.Exp)
                            if kind == "mask":
                                nc.vector.tensor_tensor(
                                    est[:, qsl], est[:, qsl],
                                    mask_sb[:, mid, qsl], mult)
                            nc.tensor.matmul(
                                o_ps[:, qsl],
                                lhsT=vsb[:, b * NKC + kc, :],
                                rhs=est[:, qsl],
                                start=(idx == 0), stop=(idx == len(vis) - 1))
                            if not into_acc:
                                nc.vector.tensor_tensor(
                                    acc[:, qsl], acc[:, qsl], est[:, qsl], add)
                        denom = dnp.tile([128, QG], F16, tag="dn")
                        nc.gpsimd.partition_all_reduce(
                            denom, acc, 128, bass_isa.ReduceOp.add)
                        rec = rcp.tile([128, QG], F32, tag="rc")
                        nc.vector.reciprocal(rec, denom)
                        nc.vector.tensor_tensor(
                            outT[hh][:, qbase:qbase + QG], o_ps, rec, mult)

                    # P3 for this window's 4 token chunks
                    for tloc in range(QG // 128):
                        tch = (b * T + g * QG) // 128 + tloc
                        panel = panp.tile([128, D], F16, tag="panel")
                        for et in range(D // 512):
                            ps = pps.tile([128, 512], F32, tag="p3")
                            for dc in range(HL):
                                nc.tensor.matmul(
                                    ps,
                                    lhsT=outT[dc][:,
                                                  tch * 128:(tch + 1) * 128],
                                    rhs=wo[dc][:, et * 512:(et + 1) * 512],
                                    start=(dc == 0), stop=(dc == HL - 1))
                            # alternate eviction engine so neither ACT nor
                            # DVE saturates; DMA each half once its 4
                            # evictions are emitted (smaller tail).
                            psl = panel[:, et * 512:(et + 1) * 512]
                            if et % 2 == 0:
                                nc.scalar.copy(psl, ps)
                            else:
                                nc.vector.tensor_copy(psl, ps)
                            if et % 4 == 3:
                                half = et // 4
                                nc.sync.dma_start(
                                    out=out_d[tch * 128:(tch + 1) * 128,
                                              half * 2048:(half + 1) * 2048],
                                    in_=panel[:, half * 2048:(half + 1) * 2048])

    nc.finalize()
    return nc, nmask


_CACHE = {}


def _get_nc(window: int):
    if window not in _CACHE:
        _CACHE[window] = build_nc(window)
    return _CACHE[window]


LAST_RESULTS = None


def kernel(x, w_qkv, w_o, window_size, _trace=False):
    window = int(window_size)
    nc, nmask = _get_nc(window)
    _, keys = _mask_plan(window)
    masks = _build_masks(window, keys)

    xT = np.ascontiguousarray(x.reshape(TOK, D).T).astype(F16NP)

    inv = 1.0 / (THETA ** (np.arange(0, HD, 2, dtype=np.float64) / HD))
    freqs = np.arange(T, dtype=np.float64)[:, None] * inv[None, :]  # [T, 64]
    cosH = np.repeat(np.cos(freqs).T, 2, axis=0).astype(F16NP)  # [128, T]
    sign = np.where(np.arange(HD) % 2 == 0, -1.0, 1.0)[:, None]
    sinH = (np.repeat(np.sin(freqs).T, 2, axis=0) * sign).astype(F16NP)

    in_maps = []
    for c in range(NCORES):
        wq = w_qkv[QROWS * c:QROWS * (c + 1)] * SCALE
        wk = w_qkv[H * HD + HD * c: H * HD + HD * (c + 1)]
        wv = w_qkv[H * HD + G * HD + HD * c: H * HD + G * HD + HD * (c + 1)]
        wqkvT = np.ascontiguousarray(
            np.concatenate([wq, wk, wv], axis=0).T).astype(F16NP)
        woT = np.ascontiguousarray(
            w_o[:, QROWS * c:QROWS * (c + 1)].T).astype(F16NP)
        in_maps.append({
            "xT": xT, "wqkvT": wqkvT, "woT": woT,
            "cosH": cosH, "sinH": sinH, "masks": masks.astype(F16NP),
        })

    from concourse.bass_utils import run_bass_kernel_spmd
    res = run_bass_kernel_spmd(nc, in_maps, core_ids=list(range(NCORES)),
                               trace=_trace)
    global LAST_RESULTS
    LAST_RESULTS = res
    acc = res.results[0]["out"].astype(np.float32)
    for c in range(1, NCORES):
        acc += res.results[c]["out"]
    return acc.reshape(B, T, D)
